# revision 29
# baseline (speedup 1.0000x reference)
"""Trainium2 Bass kernel for nn_ArthDenseCalcToDenseBlock.

The reference is a 256-step sequential scan per batch row, but the state
machine freezes at the first valid operator token (the `meet` gate), so the
whole scan collapses to closed-form masked reductions along the sequence
axis, computed per row with DVE prefix-scan instructions:

  mpre[j] = running-max of (valid-op mask)        -> first-op one-hot, met
  csuf[j] = reverse running-sum of (number mask)  -> last / 2nd-last number
                                                     one-hots via == 1 / == 2
  h0,h1 and the operator channel values are gathered with masked-sum
  accumulations; one predicated scatter writes the result back.

Data parallel over batch: 4096 rows -> 8 cores x 512 rows -> 2 halves of
[128, 2x256] merged tiles per core. trans_op is host-relayouted into 7
contiguous channel planes so every channel op is a contiguous 2D access.
Mask tensors are bf16 (0/1 and small counts are exact); trans_op values and
trans_dense stay f32 so argmax/select semantics match the reference
bit-exactly. Work is spread across DVE / GpSimd / ACT.
"""

from contextlib import ExitStack

import numpy as np

import concourse.bacc as bacc
import concourse.mybir as mybir
import concourse.tile as tile
from concourse.bass_utils import run_bass_kernel_spmd

F32 = mybir.dt.float32
BF16 = mybir.dt.bfloat16
U8 = mybir.dt.uint8
OP = mybir.AluOpType
ACTF = mybir.ActivationFunctionType

B, S, NOPS = 4096, 256, 7
NCORES = 8
BS = B // NCORES          # rows per core (512)
P = 128                   # partitions
NT = BS // P              # row-tiles per core (4)
NH = 2                    # halves per core
TPH = NT // NH            # row-tiles per half (2)
W = TPH * S               # free width of a merged half (512)


def _build_nc(sp_zero: bool):
    nc = bacc.Bacc("TRN2", target_bir_lowering=False, debug=False)

    tv_d = nc.dram_tensor("tv", [BS, S], F32, kind="ExternalInput")
    td_d = nc.dram_tensor("td", [BS, S], F32, kind="ExternalInput")
    # channel planes: op[c, row, s]
    op_d = nc.dram_tensor("op", [NOPS, BS, S], F32, kind="ExternalInput")
    fv_d = nc.dram_tensor("fv", [P, 2 * NT], F32, kind="ExternalInput")
    act_d = nc.dram_tensor("act2", [W], F32, kind="ExternalInput")

    tvo_d = nc.dram_tensor("tv_out", [BS, S], F32, kind="ExternalOutput")
    tdo_d = nc.dram_tensor("td_out", [BS, S], F32, kind="ExternalOutput")
    io_d = nc.dram_tensor("iffiv", [P * 2 * NT], F32, kind="ExternalOutput")

    with tile.TileContext(nc) as tc, ExitStack() as ctx:
        cpool = ctx.enter_context(tc.tile_pool(name="consts", bufs=1))
        io_pool = ctx.enter_context(tc.tile_pool(name="io", bufs=NH + 1))
        op_pool = ctx.enter_context(tc.tile_pool(name="op", bufs=2))
        work = ctx.enter_context(tc.tile_pool(name="work", bufs=2))
        gsc = ctx.enter_context(tc.tile_pool(name="gsc", bufs=12))
        sm = ctx.enter_context(tc.tile_pool(name="small", bufs=1))

        if not sp_zero:
            crow = cpool.tile([P, W], F32)
            nc.sync.dma_start(crow[0:1, :],
                              act_d.ap().rearrange("(o s) -> o s", o=1))
            actf = cpool.tile([P, W], F32)
            nc.gpsimd.partition_broadcast(actf[:], crow[0:1, :])
            act_bc = cpool.tile([P, W], BF16)
            nc.vector.tensor_copy(act_bc[:], actf[:])
        zero_bc = cpool.tile([P, S], F32)
        nc.vector.memset(zero_bc[:], 0.0)

        # ---- per-row gates [P, NT] (f cols 0..NT-1, g cols NT..2NT-1)
        fv = sm.tile([P, 2 * NT], F32)
        nc.gpsimd.dma_start(fv[:], fv_d[:, :])
        fin_t = fv[:, 0:NT]
        val_t = fv[:, NT : 2 * NT]
        omf = sm.tile([P, NT], F32)   # 1 - f
        nc.gpsimd.tensor_scalar(omf[:], fin_t, -1.0, 1.0, op0=OP.mult, op1=OP.add)
        gate = sm.tile([P, NT], F32)  # (1 - f) * g
        nc.gpsimd.tensor_mul(gate[:], omf[:], val_t)

        iffiv = sm.tile([P, 2 * NT], F32)
        # batched per-core scalars (columns = row-tile index 0..NT-1)
        h0_a = sm.tile([P, NT], F32)
        h1_a = sm.tile([P, NT], F32)
        enc_a = sm.tile([P, NT], F32)
        fire_a = sm.tile([P, NT], F32)
        r_a = sm.tile([P, NT], F32)

        half_state = []

        for h in range(NH):
            rows = slice(h * TPH * P, (h + 1) * TPH * P)
            tvt = io_pool.tile([P, W], F32, tag="tvt", name=f"tvt{h}")
            tdt = io_pool.tile([P, W], F32, tag="tdt", name=f"tdt{h}")
            chs = [None] * NOPS
            # pair-feeding channels on different queues so each tree leaf's
            # two inputs arrive in parallel
            qeng = {1: nc.sync, 2: nc.scalar, 3: nc.sync, 4: nc.scalar,
                    5: nc.sync, 6: nc.scalar, 0: nc.scalar}
            for c in [1, 2, 3, 4, 5, 6, 0]:
                cht = op_pool.tile([P, W], F32, tag=f"ch{c}", name=f"ch{c}_{h}")
                qeng[c].dma_start(
                    cht[:].rearrange("p (t s) -> p t s", t=TPH),
                    op_d[c, rows, :].rearrange("(t p) s -> p t s", p=P))
                chs[c] = cht
            nc.gpsimd.dma_start(
                tvt[:].rearrange("p (t s) -> p t s", t=TPH),
                tv_d[rows, :].rearrange("(t p) s -> p t s", p=P))
            nc.sync.dma_start(
                tdt[:].rearrange("p (t s) -> p t s", t=TPH),
                td_d[rows, :].rearrange("(t p) s -> p t s", p=P))

            # channel max over 1..6 via TT tree (DVE/ACT; Pool has no max)
            a1 = work.tile([P, W], F32, tag="a1", name=f"a1_{h}")
            nc.vector.tensor_max(a1[:], chs[1][:], chs[2][:])
            a2 = work.tile([P, W], F32, tag="a2", name=f"a2_{h}")
            nc.any.tensor_max(a2[:], chs[3][:], chs[4][:])
            a3 = work.tile([P, W], F32, tag="a3", name=f"a3_{h}")
            nc.any.tensor_max(a3[:], chs[5][:], chs[6][:])
            b1 = work.tile([P, W], F32, tag="b1", name=f"b1_{h}")
            nc.any.tensor_max(b1[:], a1[:], a2[:])
            m6 = work.tile([P, W], F32, tag="m6", name=f"m6_{h}")
            nc.vector.tensor_max(m6[:], b1[:], a3[:])
            is_op = work.tile([P, W], BF16, tag="isop", name=f"isop_{h}")
            nc.any.tensor_tensor(is_op[:], m6[:], chs[0][:], op=OP.is_gt)

            # argmax channel index, packed as 5 first-wins comparison bits
            s1 = work.tile([P, W], BF16, tag="s1", name=f"s1_{h}")
            nc.vector.tensor_tensor(s1[:], chs[1][:], chs[2][:], op=OP.is_ge)
            s2 = work.tile([P, W], BF16, tag="s2", name=f"s2_{h}")
            nc.any.tensor_tensor(s2[:], chs[3][:], chs[4][:], op=OP.is_ge)
            s3 = work.tile([P, W], BF16, tag="s3", name=f"s3_{h}")
            nc.any.tensor_tensor(s3[:], chs[5][:], chs[6][:], op=OP.is_ge)
            t1 = work.tile([P, W], BF16, tag="t1", name=f"t1_{h}")
            nc.any.tensor_tensor(t1[:], a1[:], a2[:], op=OP.is_ge)
            rt = work.tile([P, W], BF16, tag="rt", name=f"rt_{h}")
            nc.vector.tensor_tensor(rt[:], b1[:], a3[:], op=OP.is_ge)
            pk1 = work.tile([P, W], BF16, tag="pk1", name=f"pk1_{h}")
            nc.vector.scalar_tensor_tensor(
                pk1[:], s2[:], 2.0, s1[:], op0=OP.mult, op1=OP.add)
            pk2 = work.tile([P, W], BF16, tag="pk2", name=f"pk2_{h}")
            nc.vector.scalar_tensor_tensor(
                pk2[:], s3[:], 4.0, pk1[:], op0=OP.mult, op1=OP.add)
            pk3 = work.tile([P, W], BF16, tag="pk3", name=f"pk3_{h}")
            nc.vector.scalar_tensor_tensor(
                pk3[:], t1[:], 8.0, pk2[:], op0=OP.mult, op1=OP.add)
            enc = work.tile([P, W], BF16, tag="enc", name=f"enc_{h}")
            nc.vector.scalar_tensor_tensor(
                enc[:], rt[:], 16.0, pk3[:], op0=OP.mult, op1=OP.add)

            # valid-token mask (f/g gate folded into per-row scalars later)
            tvb = tvt
            if not sp_zero:
                tva = work.tile([P, W], BF16, tag="tva", name=f"tva_{h}")
                nc.vector.tensor_mul(tva[:], tvt[:], act_bc[:])
                tvb = tva
            vop = work.tile([P, W], BF16, tag="vop", name=f"vop_{h}")
            nc.vector.tensor_mul(vop[:], tvb[:], is_op[:])

            # mpre[j] = running max of vop (per 256-tile)
            mpre = work.tile([P, W], BF16, tag="mpre", name=f"mpre_{h}")
            for t in range(TPH):
                ts = slice(t * S, (t + 1) * S)
                nc.vector.tensor_tensor_scan(
                    mpre[:][:, ts], vop[:][:, ts], vop[:][:, ts], 0.0,
                    op0=OP.max, op1=OP.max)

            # nshx[1+j] = (mpre[j] != 1); nshx[0] = 1
            nshx = work.tile([P, W + 1], BF16, tag="nshx", name=f"nshx_{h}")
            nc.vector.tensor_single_scalar(
                nshx[:][:, 1 : W + 1], mpre[:], 1.0, op=OP.not_equal)
            nc.vector.memset(nshx[:][:, 0:1], 1.0)
            # first-op one-hot: vop & ~meet_exclusive (fix tile boundary col)
            ohi = work.tile([P, W], BF16, tag="ohi", name=f"ohi_{h}")
            nc.vector.tensor_mul(ohi[:], nshx[:][:, 0:W], vop[:])
            for t in range(1, TPH):
                nc.vector.tensor_copy(ohi[:][:, t * S : t * S + 1],
                                      vop[:][:, t * S : t * S + 1])
            # numbers before i*: tvb & ~mpre (inclusive)
            vnum = work.tile([P, W], BF16, tag="vnum", name=f"vnum_{h}")
            nc.vector.tensor_mul(vnum[:], nshx[:][:, 1 : W + 1], tvb[:])

            # csuf[j] = inclusive suffix count of vnum (reverse scan per tile)
            csuf = work.tile([P, W], BF16, tag="csuf", name=f"csuf_{h}")
            for t in range(TPH):
                ts = slice(t * S, (t + 1) * S)
                vr = vnum[:][:, ts][:, ::-1]
                nc.vector.tensor_tensor_scan(
                    csuf[:][:, ts][:, ::-1], vr, vr, 0.0, op0=OP.add, op1=OP.max)

            # last & second-to-last number one-hots (constant compares)
            e0 = work.tile([P, W], BF16, tag="e0", name=f"e0_{h}")
            nc.vector.tensor_single_scalar(e0[:], csuf[:], 1.0, op=OP.is_equal)
            ohp0 = work.tile([P, W], BF16, tag="ohp0", name=f"ohp0_{h}")
            nc.vector.tensor_mul(ohp0[:], e0[:], vnum[:])
            e1 = work.tile([P, W], BF16, tag="e1", name=f"e1_{h}")
            nc.vector.tensor_single_scalar(e1[:], csuf[:], 2.0, op=OP.is_equal)
            ohp1 = work.tile([P, W], BF16, tag="ohp1", name=f"ohp1_{h}")
            nc.vector.tensor_mul(ohp1[:], e1[:], vnum[:])

            # per-row scalars for this half
            met2 = mpre[:][:, S - 1 :: S]       # [P, TPH] bf16
            total2 = csuf[:][:, 0::S]           # [P, TPH] bf16
            gate2 = gate[:, h * TPH : (h + 1) * TPH]
            acols = slice(h * TPH, (h + 1) * TPH)

            cnt1 = sm.tile([P, TPH], F32, name=f"cnt1_{h}")
            nc.gpsimd.tensor_scalar(cnt1[:], total2, 0.5, None, op0=OP.is_gt)
            cnt2 = sm.tile([P, TPH], F32, name=f"cnt2_{h}")
            nc.gpsimd.tensor_scalar(cnt2[:], total2, 1.5, None, op0=OP.is_gt)
            metg = sm.tile([P, TPH], F32, name=f"metg_{h}")
            nc.gpsimd.tensor_mul(metg[:], met2, gate2)
            fire2 = fire_a[:, acols]
            nc.gpsimd.tensor_mul(fire2, metg[:], cnt2[:])

            # masked-sum gathers (exact: at most one nonzero term)
            def gather_dve(dst_col, src_ap, mask_ap):
                scr = gsc.tile([P, S], F32, tag="gscr", name="gscr")
                nc.vector.scalar_tensor_tensor(
                    scr[:], src_ap, 0.0, mask_ap, op0=OP.bypass, op1=OP.mult,
                    accum_out=dst_col)

            for t in range(TPH):
                ts = slice(t * S, (t + 1) * S)
                col = slice(h * TPH + t, h * TPH + t + 1)
                tds = tdt[:][:, ts]
                ohp0s, ohp1s, ohis = ohp0[:][:, ts], ohp1[:][:, ts], ohi[:][:, ts]
                gather_dve(h0_a[:, col], tds, ohp0s)
                gather_dve(h1_a[:, col], tds, ohp1s)
                gather_dve(enc_a[:, col], enc[:][:, ts], ohis)

            # iv / iff (gps smalls)
            nmet = sm.tile([P, TPH], F32, name=f"nmet_{h}")
            nc.gpsimd.tensor_scalar(nmet[:], metg[:], -1.0, 1.0,
                                    op0=OP.mult, op1=OP.add)
            g2 = val_t[:, acols]
            nmg = sm.tile([P, TPH], F32, name=f"nmg_{h}")
            nc.gpsimd.tensor_mul(nmg[:], nmet[:], g2)
            nc.gpsimd.tensor_add(iffiv[:, NT + h * TPH : NT + (h + 1) * TPH],
                                 fire2, nmg[:])
            ncnt2 = sm.tile([P, TPH], F32, name=f"ncnt2_{h}")
            nc.gpsimd.tensor_scalar(ncnt2[:], cnt2[:], -1.0, 1.0,
                                    op0=OP.mult, op1=OP.add)
            q1 = sm.tile([P, TPH], F32, name=f"q1_{h}")
            nc.gpsimd.tensor_mul(q1[:], nmg[:], cnt1[:])
            ifu = sm.tile([P, TPH], F32, name=f"ifu_{h}")
            nc.gpsimd.tensor_mul(ifu[:], q1[:], ncnt2[:])
            q2 = sm.tile([P, TPH], F32, name=f"q2_{h}")
            nc.gpsimd.tensor_mul(q2[:], omf[:, acols], ifu[:])
            nc.gpsimd.tensor_add(iffiv[:, h * TPH : (h + 1) * TPH],
                                 q2[:], fin_t[:, acols])

            # tv[p1] <- 0, tv[i*] <- 0 where fire  (doesn't need r)
            u1 = work.tile([P, W], BF16, tag="u1", name=f"u1_{h}")
            nc.gpsimd.tensor_add(u1[:], ohp1[:], ohi[:])
            for t in range(TPH):
                ts = slice(t * S, (t + 1) * S)
                fcol = fire_a[:, h * TPH + t : h * TPH + t + 1]
                nw1 = gsc.tile([P, S], F32, tag="nw1", name="nw1")
                nc.vector.tensor_scalar(nw1[:], u1[:][:, ts], fcol, 1.0,
                                        op0=OP.mult, op1=OP.not_equal)
                nc.vector.tensor_mul(tvt[:][:, ts], tvt[:][:, ts], nw1[:])
            nc.sync.dma_start(
                tvo_d[rows, :].rearrange("(t p) s -> p t s", p=P),
                tvt[:].rearrange("p (t s) -> p t s", t=TPH))

            half_state.append((tdt, ohp0, rows))

        # ---- batched r computation ([P, NT], one Ln + one Exp table load)
        radd = sm.tile([P, NT], F32)
        nc.gpsimd.tensor_add(radd[:], h1_a[:], h0_a[:])
        rsub = sm.tile([P, NT], F32)
        nc.gpsimd.tensor_sub(rsub[:], h1_a[:], h0_a[:])
        rmul = sm.tile([P, NT], F32)
        nc.gpsimd.tensor_mul(rmul[:], h1_a[:], h0_a[:])
        den = sm.tile([P, NT], F32)
        nc.gpsimd.tensor_scalar_add(den[:], h0_a[:], 1e-7)
        rec = sm.tile([P, NT], F32)
        nc.vector.reciprocal(rec[:], den[:])
        rdiv = sm.tile([P, NT], F32)
        nc.gpsimd.tensor_mul(rdiv[:], h1_a[:], rec[:])
        base = sm.tile([P, NT], F32)
        nc.gpsimd.tensor_scalar_max(base[:], h1_a[:], 1e-7)
        lg = sm.tile([P, NT], F32)
        nc.scalar.activation(lg[:], base[:], ACTF.Ln)
        pm = sm.tile([P, NT], F32)
        nc.gpsimd.tensor_mul(pm[:], lg[:], h0_a[:])
        rpow = sm.tile([P, NT], F32)
        nc.scalar.activation(rpow[:], pm[:], ACTF.Exp)

        def bit_split(src_t, thr, wgt, nlo, nbit):
            b = sm.tile([P, NT], F32, name=nbit)
            nc.gpsimd.tensor_scalar(b[:], src_t, thr, None, op0=OP.is_gt)
            m = sm.tile([P, NT], F32, name=nbit + "m")
            nc.gpsimd.tensor_scalar(m[:], b[:], wgt, None, op0=OP.mult)
            lo = sm.tile([P, NT], F32, name=nlo)
            nc.gpsimd.tensor_sub(lo[:], src_t, m[:])
            return b, lo

        rt_b, rr1 = bit_split(enc_a[:], 15.5, 16.0, "rr1", "rtb")
        t1_b, rr2 = bit_split(rr1[:], 7.5, 8.0, "rr2", "t1b")
        s3_b, rr3 = bit_split(rr2[:], 3.5, 4.0, "rr3", "s3b")
        s2_b, s1_b = bit_split(rr3[:], 1.5, 2.0, "s1b", "s2b")
        i12 = sm.tile([P, NT], F32)
        nc.gpsimd.tensor_scalar(i12[:], s1_b[:], -1.0, 2.0, op0=OP.mult, op1=OP.add)
        i34 = sm.tile([P, NT], F32)
        nc.gpsimd.tensor_scalar(i34[:], s2_b[:], -1.0, 4.0, op0=OP.mult, op1=OP.add)
        i56 = sm.tile([P, NT], F32)
        nc.gpsimd.tensor_scalar(i56[:], s3_b[:], -1.0, 6.0, op0=OP.mult, op1=OP.add)
        dl = sm.tile([P, NT], F32)
        nc.gpsimd.tensor_sub(dl[:], i12[:], i34[:])
        el = sm.tile([P, NT], F32)
        nc.gpsimd.tensor_mul(el[:], t1_b[:], dl[:])
        left = sm.tile([P, NT], F32)
        nc.gpsimd.tensor_add(left[:], el[:], i34[:])
        dr = sm.tile([P, NT], F32)
        nc.gpsimd.tensor_sub(dr[:], left[:], i56[:])
        er = sm.tile([P, NT], F32)
        nc.gpsimd.tensor_mul(er[:], rt_b[:], dr[:])
        cstar = sm.tile([P, NT], F32)
        nc.gpsimd.tensor_add(cstar[:], er[:], i56[:])

        nc.vector.memset(r_a[:], 0.0)
        ohc = sm.tile([P, NT], U8, name="ohc", bufs=2)
        for c, res in zip(range(2, 7), [radd, rsub, rmul, rdiv, rpow]):
            nc.vector.tensor_single_scalar(ohc[:], cstar[:], float(c),
                                           op=OP.is_equal)
            nc.vector.copy_predicated(r_a[:], ohc[:], res[:])

        nc.gpsimd.dma_start(io_d.ap().rearrange("(p q) -> p q", p=P), iffiv[:])

        # ---- td scatter emitted after both halves' compute so each engine's
        # in-order stream can overlap half0's r-wait with half1's work
        for h, (tdt, ohp0, rows) in enumerate(half_state):
            for t in range(TPH):
                ts = slice(t * S, (t + 1) * S)
                cidx = h * TPH + t
                fcol = fire_a[:, cidx : cidx + 1]
                rcol = r_a[:, cidx : cidx + 1]
                w0 = gsc.tile([P, S], U8, tag="w0", name="w0")
                nc.vector.tensor_scalar(w0[:], ohp0[:][:, ts], fcol, None,
                                        op0=OP.mult)
                rb = gsc.tile([P, S], F32, tag="rb", name="rb")
                nc.vector.tensor_scalar(rb[:], zero_bc[:], 0.0, rcol,
                                        op0=OP.mult, op1=OP.add)
                nc.vector.copy_predicated(tdt[:][:, ts], w0[:], rb[:])
            # one DMA per row-tile, alternating queues to drain in parallel
            for t in range(TPH):
                qout = nc.sync if (h * TPH + t) % 2 == 0 else nc.scalar
                rtile = slice((h * TPH + t) * P, (h * TPH + t + 1) * P)
                qout.dma_start(tdo_d[rtile, :],
                               tdt[:][:, t * S : (t + 1) * S])

    nc.compile()
    return nc


_NC_CACHE = {}


def _get_nc(sp_zero: bool = True):
    if sp_zero not in _NC_CACHE:
        _NC_CACHE[sp_zero] = _build_nc(sp_zero)
    return _NC_CACHE[sp_zero]


def _make_in_maps(trans_valid, trans_dense, trans_op, if_finished, if_valid,
                  start_pos):
    tv = np.ascontiguousarray(np.asarray(trans_valid, np.float32))
    td = np.ascontiguousarray(np.asarray(trans_dense, np.float32))
    # layout-only: channel planes [7, B, S]
    op = np.ascontiguousarray(
        np.asarray(trans_op, np.float32).transpose(2, 0, 1))
    fin = np.asarray(if_finished, np.float32)
    val = np.asarray(if_valid, np.float32)
    sp = int(start_pos)
    act2 = np.ascontiguousarray(
        np.tile((np.arange(S) >= sp).astype(np.float32), TPH))
    in_maps = []
    for c in range(NCORES):
        rows = slice(c * BS, (c + 1) * BS)
        # fv[p, t] = fin[t*128+p] ; fv[p, NT+t] = val[t*128+p]
        fvc = np.concatenate(
            [fin[rows].reshape(NT, P).T, val[rows].reshape(NT, P).T], axis=1)
        in_maps.append({
            "tv": tv[rows], "td": td[rows],
            "op": np.ascontiguousarray(op[:, rows, :]),
            "fv": np.ascontiguousarray(fvc), "act2": act2,
        })
    return in_maps


def _unpack_outs(outs, trans_op):
    tv_out = np.concatenate([outs[c]["tv_out"] for c in range(NCORES)], axis=0)
    td_out = np.concatenate([outs[c]["td_out"] for c in range(NCORES)], axis=0)
    iff = np.empty(B, np.float32)
    iv = np.empty(B, np.float32)
    for c in range(NCORES):
        arr = outs[c]["iffiv"].reshape(P, 2 * NT)
        rows = slice(c * BS, (c + 1) * BS)
        iff[rows] = arr[:, 0:NT].T.reshape(BS)
        iv[rows] = arr[:, NT : 2 * NT].T.reshape(BS)
    return tv_out, td_out, np.asarray(trans_op, np.float32), iff, iv


def kernel(trans_valid, trans_dense, trans_op, if_finished, if_valid, start_pos):
    nc = _get_nc(int(start_pos) == 0)
    in_maps = _make_in_maps(trans_valid, trans_dense, trans_op, if_finished,
                            if_valid, start_pos)
    res = run_bass_kernel_spmd(nc, in_maps, core_ids=list(range(NCORES)))
    return _unpack_outs(res.results, trans_op)


# revision 31
# speedup vs baseline: 1.1592x; 1.1592x over previous
"""Trainium2 Bass kernel for nn_ArthDenseCalcToDenseBlock.

The reference is a 256-step sequential scan per batch row, but the state
machine freezes at the first valid operator token (the `meet` gate), so the
whole scan collapses to closed-form masked reductions along the sequence
axis, computed per row with DVE prefix-scan instructions:

  mpre[j] = running-max of (valid-op mask)        -> first-op one-hot, met
  csuf[j] = reverse running-sum of (number mask)  -> last / 2nd-last number
                                                     one-hots via == 1 / == 2
  h0,h1 and the operator channel values are gathered with masked-sum
  accumulations; one predicated scatter writes the result back.

Data parallel over batch: 4096 rows -> 8 cores x 512 rows -> 2 halves of
[128, 2x256] merged tiles per core. trans_op is host-relayouted into 7
contiguous channel planes so every channel op is a contiguous 2D access.
Mask tensors are bf16 (0/1 and small counts are exact); trans_op values and
trans_dense stay f32 so argmax/select semantics match the reference
bit-exactly. Work is spread across DVE / GpSimd / ACT.
"""

from contextlib import ExitStack

import numpy as np

import concourse.bacc as bacc
import concourse.mybir as mybir
import concourse.tile as tile
from concourse.bass_utils import run_bass_kernel_spmd

F32 = mybir.dt.float32
BF16 = mybir.dt.bfloat16
U8 = mybir.dt.uint8
OP = mybir.AluOpType
ACTF = mybir.ActivationFunctionType

B, S, NOPS = 4096, 256, 7
NCORES = 8
BS = B // NCORES          # rows per core (512)
P = 128                   # partitions
NT = BS // P              # row-tiles per core (4)
NH = 2                    # halves per core
TPH = NT // NH            # row-tiles per half (2)
W = TPH * S               # free width of a merged half (512)


def _build_nc(sp_zero: bool):
    nc = bacc.Bacc("TRN2", target_bir_lowering=False, debug=False)

    tv_d = nc.dram_tensor("tv", [BS, S], F32, kind="ExternalInput")
    td_d = nc.dram_tensor("td", [BS, S], F32, kind="ExternalInput")
    # channel planes: op[c, row, s]
    op_d = nc.dram_tensor("op", [NOPS, BS, S], F32, kind="ExternalInput")
    fv_d = nc.dram_tensor("fv", [P, 2 * NT], F32, kind="ExternalInput")
    act_d = nc.dram_tensor("act2", [W], F32, kind="ExternalInput")

    tvo_d = nc.dram_tensor("tv_out", [BS, S], F32, kind="ExternalOutput")
    tdo_d = nc.dram_tensor("td_out", [BS, S], F32, kind="ExternalOutput")
    io_d = nc.dram_tensor("iffiv", [P * 2 * NT], F32, kind="ExternalOutput")

    with tile.TileContext(nc) as tc, ExitStack() as ctx:
        cpool = ctx.enter_context(tc.tile_pool(name="consts", bufs=1))
        io_pool = ctx.enter_context(tc.tile_pool(name="io", bufs=NH + 1))
        op_pool = ctx.enter_context(tc.tile_pool(name="op", bufs=2))
        work = ctx.enter_context(tc.tile_pool(name="work", bufs=2))
        gsc = ctx.enter_context(tc.tile_pool(name="gsc", bufs=12))
        sm = ctx.enter_context(tc.tile_pool(name="small", bufs=1))

        if not sp_zero:
            crow = cpool.tile([P, W], F32)
            nc.sync.dma_start(crow[0:1, :],
                              act_d.ap().rearrange("(o s) -> o s", o=1))
            actf = cpool.tile([P, W], F32)
            nc.gpsimd.partition_broadcast(actf[:], crow[0:1, :])
            act_bc = cpool.tile([P, W], BF16)
            nc.vector.tensor_copy(act_bc[:], actf[:])
        zero_bc = cpool.tile([P, S], F32)
        nc.vector.memset(zero_bc[:], 0.0)

        # ---- per-row gates [P, NT] (f cols 0..NT-1, g cols NT..2NT-1)
        fv = sm.tile([P, 2 * NT], F32)
        nc.gpsimd.dma_start(fv[:], fv_d[:, :])
        fin_t = fv[:, 0:NT]
        val_t = fv[:, NT : 2 * NT]
        omf = sm.tile([P, NT], F32)   # 1 - f
        nc.gpsimd.tensor_scalar(omf[:], fin_t, -1.0, 1.0, op0=OP.mult, op1=OP.add)
        gate = sm.tile([P, NT], F32)  # (1 - f) * g
        nc.gpsimd.tensor_mul(gate[:], omf[:], val_t)

        iffiv = sm.tile([P, 2 * NT], F32)
        # batched per-core scalars (columns = row-tile index 0..NT-1)
        h0_a = sm.tile([P, NT], F32)
        h1_a = sm.tile([P, NT], F32)
        enc_a = sm.tile([P, NT], F32)
        fire_a = sm.tile([P, NT], F32)
        r_a = sm.tile([P, NT], F32)

        half_state = []

        for h in range(NH):
            rows = slice(h * TPH * P, (h + 1) * TPH * P)
            tvt = io_pool.tile([P, W], F32, tag="tvt", name=f"tvt{h}")
            tdt = io_pool.tile([P, W], F32, tag="tdt", name=f"tdt{h}")
            chs = [None] * NOPS
            # pair-feeding channels on different queues so each tree leaf's
            # two inputs arrive in parallel
            qeng = {1: nc.sync, 2: nc.scalar, 3: nc.sync, 4: nc.scalar,
                    5: nc.sync, 6: nc.scalar, 0: nc.scalar}
            for c in [1, 2, 3, 4, 5, 6, 0]:
                cht = op_pool.tile([P, W], F32, tag=f"ch{c}", name=f"ch{c}_{h}")
                qeng[c].dma_start(
                    cht[:].rearrange("p (t s) -> p t s", t=TPH),
                    op_d[c, rows, :].rearrange("(t p) s -> p t s", p=P))
                chs[c] = cht
            nc.gpsimd.dma_start(
                tvt[:].rearrange("p (t s) -> p t s", t=TPH),
                tv_d[rows, :].rearrange("(t p) s -> p t s", p=P))
            nc.sync.dma_start(
                tdt[:].rearrange("p (t s) -> p t s", t=TPH),
                td_d[rows, :].rearrange("(t p) s -> p t s", p=P))

            # channel max over 1..6 via TT tree (DVE/ACT; Pool has no max)
            a1 = work.tile([P, W], F32, tag="a1", name=f"a1_{h}")
            nc.vector.tensor_max(a1[:], chs[1][:], chs[2][:])
            a2 = work.tile([P, W], F32, tag="a2", name=f"a2_{h}")
            nc.any.tensor_max(a2[:], chs[3][:], chs[4][:])
            a3 = work.tile([P, W], F32, tag="a3", name=f"a3_{h}")
            nc.any.tensor_max(a3[:], chs[5][:], chs[6][:])
            b1 = work.tile([P, W], F32, tag="b1", name=f"b1_{h}")
            nc.any.tensor_max(b1[:], a1[:], a2[:])
            m6 = work.tile([P, W], F32, tag="m6", name=f"m6_{h}")
            nc.vector.tensor_max(m6[:], b1[:], a3[:])
            is_op = work.tile([P, W], BF16, tag="isop", name=f"isop_{h}")
            nc.any.tensor_tensor(is_op[:], m6[:], chs[0][:], op=OP.is_gt)

            # argmax channel index, packed as 5 first-wins comparison bits
            s1 = work.tile([P, W], BF16, tag="s1", name=f"s1_{h}")
            nc.vector.tensor_tensor(s1[:], chs[1][:], chs[2][:], op=OP.is_ge)
            s2 = work.tile([P, W], BF16, tag="s2", name=f"s2_{h}")
            nc.any.tensor_tensor(s2[:], chs[3][:], chs[4][:], op=OP.is_ge)
            s3 = work.tile([P, W], BF16, tag="s3", name=f"s3_{h}")
            nc.any.tensor_tensor(s3[:], chs[5][:], chs[6][:], op=OP.is_ge)
            t1 = work.tile([P, W], BF16, tag="t1", name=f"t1_{h}")
            nc.any.tensor_tensor(t1[:], a1[:], a2[:], op=OP.is_ge)
            rt = work.tile([P, W], BF16, tag="rt", name=f"rt_{h}")
            nc.vector.tensor_tensor(rt[:], b1[:], a3[:], op=OP.is_ge)
            pk1 = work.tile([P, W], BF16, tag="pk1", name=f"pk1_{h}")
            nc.vector.scalar_tensor_tensor(
                pk1[:], s2[:], 2.0, s1[:], op0=OP.mult, op1=OP.add)
            pk2 = work.tile([P, W], BF16, tag="pk2", name=f"pk2_{h}")
            nc.vector.scalar_tensor_tensor(
                pk2[:], s3[:], 4.0, pk1[:], op0=OP.mult, op1=OP.add)
            pk3 = work.tile([P, W], BF16, tag="pk3", name=f"pk3_{h}")
            nc.vector.scalar_tensor_tensor(
                pk3[:], t1[:], 8.0, pk2[:], op0=OP.mult, op1=OP.add)
            enc = work.tile([P, W], BF16, tag="enc", name=f"enc_{h}")
            nc.vector.scalar_tensor_tensor(
                enc[:], rt[:], 16.0, pk3[:], op0=OP.mult, op1=OP.add)

            # valid-token mask (f/g gate folded into per-row scalars later)
            tvb = tvt
            if not sp_zero:
                tva = work.tile([P, W], BF16, tag="tva", name=f"tva_{h}")
                nc.vector.tensor_mul(tva[:], tvt[:], act_bc[:])
                tvb = tva
            vop = work.tile([P, W], BF16, tag="vop", name=f"vop_{h}")
            nc.vector.tensor_mul(vop[:], tvb[:], is_op[:])

            # mpre[j] = running max of vop (per 256-tile)
            mpre = work.tile([P, W], BF16, tag="mpre", name=f"mpre_{h}")
            for t in range(TPH):
                ts = slice(t * S, (t + 1) * S)
                nc.vector.tensor_tensor_scan(
                    mpre[:][:, ts], vop[:][:, ts], vop[:][:, ts], 0.0,
                    op0=OP.max, op1=OP.max)

            # nshx[1+j] = (mpre[j] != 1); nshx[0] = 1
            nshx = work.tile([P, W + 1], BF16, tag="nshx", name=f"nshx_{h}")
            nc.vector.tensor_single_scalar(
                nshx[:][:, 1 : W + 1], mpre[:], 1.0, op=OP.not_equal)
            nc.vector.memset(nshx[:][:, 0:1], 1.0)
            # first-op one-hot: vop & ~meet_exclusive (fix tile boundary col)
            ohi = work.tile([P, W], BF16, tag="ohi", name=f"ohi_{h}")
            nc.vector.tensor_mul(ohi[:], nshx[:][:, 0:W], vop[:])
            for t in range(1, TPH):
                nc.vector.tensor_copy(ohi[:][:, t * S : t * S + 1],
                                      vop[:][:, t * S : t * S + 1])
            # numbers before i*: tvb & ~mpre (inclusive)
            vnum = work.tile([P, W], BF16, tag="vnum", name=f"vnum_{h}")
            nc.vector.tensor_mul(vnum[:], nshx[:][:, 1 : W + 1], tvb[:])

            # csuf[j] = inclusive suffix count of vnum (reverse scan per tile)
            csuf = work.tile([P, W], BF16, tag="csuf", name=f"csuf_{h}")
            for t in range(TPH):
                ts = slice(t * S, (t + 1) * S)
                vr = vnum[:][:, ts][:, ::-1]
                nc.vector.tensor_tensor_scan(
                    csuf[:][:, ts][:, ::-1], vr, vr, 0.0, op0=OP.add, op1=OP.max)

            # last & second-to-last number one-hots (constant compares)
            e0 = work.tile([P, W], BF16, tag="e0", name=f"e0_{h}")
            nc.vector.tensor_single_scalar(e0[:], csuf[:], 1.0, op=OP.is_equal)
            ohp0 = work.tile([P, W], BF16, tag="ohp0", name=f"ohp0_{h}")
            nc.vector.tensor_mul(ohp0[:], e0[:], vnum[:])
            e1 = work.tile([P, W], BF16, tag="e1", name=f"e1_{h}")
            nc.vector.tensor_single_scalar(e1[:], csuf[:], 2.0, op=OP.is_equal)
            ohp1 = work.tile([P, W], BF16, tag="ohp1", name=f"ohp1_{h}")
            nc.vector.tensor_mul(ohp1[:], e1[:], vnum[:])

            # per-row scalars for this half
            met2 = mpre[:][:, S - 1 :: S]       # [P, TPH] bf16
            total2 = csuf[:][:, 0::S]           # [P, TPH] bf16
            gate2 = gate[:, h * TPH : (h + 1) * TPH]
            acols = slice(h * TPH, (h + 1) * TPH)

            cnt1 = sm.tile([P, TPH], F32, name=f"cnt1_{h}")
            nc.gpsimd.tensor_scalar(cnt1[:], total2, 0.5, None, op0=OP.is_gt)
            cnt2 = sm.tile([P, TPH], F32, name=f"cnt2_{h}")
            nc.gpsimd.tensor_scalar(cnt2[:], total2, 1.5, None, op0=OP.is_gt)
            metg = sm.tile([P, TPH], F32, name=f"metg_{h}")
            nc.gpsimd.tensor_mul(metg[:], met2, gate2)
            fire2 = fire_a[:, acols]
            nc.gpsimd.tensor_mul(fire2, metg[:], cnt2[:])

            # masked-sum gathers (exact: at most one nonzero term)
            def gather_dve(dst_col, src_ap, mask_ap):
                scr = gsc.tile([P, S], F32, tag="gscr", name="gscr")
                nc.vector.scalar_tensor_tensor(
                    scr[:], src_ap, 0.0, mask_ap, op0=OP.bypass, op1=OP.mult,
                    accum_out=dst_col)

            for t in range(TPH):
                ts = slice(t * S, (t + 1) * S)
                col = slice(h * TPH + t, h * TPH + t + 1)
                tds = tdt[:][:, ts]
                ohp0s, ohp1s, ohis = ohp0[:][:, ts], ohp1[:][:, ts], ohi[:][:, ts]
                gather_dve(h0_a[:, col], tds, ohp0s)
                gather_dve(h1_a[:, col], tds, ohp1s)
                gather_dve(enc_a[:, col], enc[:][:, ts], ohis)

            # iv / iff (gps smalls)
            nmet = sm.tile([P, TPH], F32, name=f"nmet_{h}")
            nc.gpsimd.tensor_scalar(nmet[:], metg[:], -1.0, 1.0,
                                    op0=OP.mult, op1=OP.add)
            g2 = val_t[:, acols]
            nmg = sm.tile([P, TPH], F32, name=f"nmg_{h}")
            nc.gpsimd.tensor_mul(nmg[:], nmet[:], g2)
            nc.gpsimd.tensor_add(iffiv[:, NT + h * TPH : NT + (h + 1) * TPH],
                                 fire2, nmg[:])
            ncnt2 = sm.tile([P, TPH], F32, name=f"ncnt2_{h}")
            nc.gpsimd.tensor_scalar(ncnt2[:], cnt2[:], -1.0, 1.0,
                                    op0=OP.mult, op1=OP.add)
            q1 = sm.tile([P, TPH], F32, name=f"q1_{h}")
            nc.gpsimd.tensor_mul(q1[:], nmg[:], cnt1[:])
            ifu = sm.tile([P, TPH], F32, name=f"ifu_{h}")
            nc.gpsimd.tensor_mul(ifu[:], q1[:], ncnt2[:])
            q2 = sm.tile([P, TPH], F32, name=f"q2_{h}")
            nc.gpsimd.tensor_mul(q2[:], omf[:, acols], ifu[:])
            nc.gpsimd.tensor_add(iffiv[:, h * TPH : (h + 1) * TPH],
                                 q2[:], fin_t[:, acols])

            # tv[p1] <- 0, tv[i*] <- 0 where fire  (doesn't need r)
            u1 = work.tile([P, W], BF16, tag="u1", name=f"u1_{h}")
            nc.gpsimd.tensor_add(u1[:], ohp1[:], ohi[:])
            for t in range(TPH):
                ts = slice(t * S, (t + 1) * S)
                fcol = fire_a[:, h * TPH + t : h * TPH + t + 1]
                nw1 = gsc.tile([P, S], F32, tag="nw1", name="nw1")
                nc.vector.tensor_scalar(nw1[:], u1[:][:, ts], fcol, 1.0,
                                        op0=OP.mult, op1=OP.not_equal)
                nc.vector.tensor_mul(tvt[:][:, ts], tvt[:][:, ts], nw1[:])
            nc.sync.dma_start(
                tvo_d[rows, :].rearrange("(t p) s -> p t s", p=P),
                tvt[:].rearrange("p (t s) -> p t s", t=TPH))

            half_state.append((tdt, ohp0, rows))

        # ---- batched r computation ([P, NT], one Ln + one Exp table load)
        radd = sm.tile([P, NT], F32)
        nc.gpsimd.tensor_add(radd[:], h1_a[:], h0_a[:])
        rsub = sm.tile([P, NT], F32)
        nc.gpsimd.tensor_sub(rsub[:], h1_a[:], h0_a[:])
        rmul = sm.tile([P, NT], F32)
        nc.gpsimd.tensor_mul(rmul[:], h1_a[:], h0_a[:])
        den = sm.tile([P, NT], F32)
        nc.gpsimd.tensor_scalar_add(den[:], h0_a[:], 1e-7)
        rec = sm.tile([P, NT], F32)
        nc.vector.reciprocal(rec[:], den[:])
        rdiv = sm.tile([P, NT], F32)
        nc.gpsimd.tensor_mul(rdiv[:], h1_a[:], rec[:])
        base = sm.tile([P, NT], F32)
        nc.gpsimd.tensor_scalar_max(base[:], h1_a[:], 1e-7)
        lg = sm.tile([P, NT], F32)
        nc.scalar.activation(lg[:], base[:], ACTF.Ln)
        pm = sm.tile([P, NT], F32)
        nc.gpsimd.tensor_mul(pm[:], lg[:], h0_a[:])
        rpow = sm.tile([P, NT], F32)
        nc.scalar.activation(rpow[:], pm[:], ACTF.Exp)

        def bit_split(src_t, thr, wgt, nlo, nbit):
            b = sm.tile([P, NT], F32, name=nbit)
            nc.gpsimd.tensor_scalar(b[:], src_t, thr, None, op0=OP.is_gt)
            m = sm.tile([P, NT], F32, name=nbit + "m")
            nc.gpsimd.tensor_scalar(m[:], b[:], wgt, None, op0=OP.mult)
            lo = sm.tile([P, NT], F32, name=nlo)
            nc.gpsimd.tensor_sub(lo[:], src_t, m[:])
            return b, lo

        rt_b, rr1 = bit_split(enc_a[:], 15.5, 16.0, "rr1", "rtb")
        t1_b, rr2 = bit_split(rr1[:], 7.5, 8.0, "rr2", "t1b")
        s3_b, rr3 = bit_split(rr2[:], 3.5, 4.0, "rr3", "s3b")
        s2_b, s1_b = bit_split(rr3[:], 1.5, 2.0, "s1b", "s2b")
        i12 = sm.tile([P, NT], F32)
        nc.gpsimd.tensor_scalar(i12[:], s1_b[:], -1.0, 2.0, op0=OP.mult, op1=OP.add)
        i34 = sm.tile([P, NT], F32)
        nc.gpsimd.tensor_scalar(i34[:], s2_b[:], -1.0, 4.0, op0=OP.mult, op1=OP.add)
        i56 = sm.tile([P, NT], F32)
        nc.gpsimd.tensor_scalar(i56[:], s3_b[:], -1.0, 6.0, op0=OP.mult, op1=OP.add)
        dl = sm.tile([P, NT], F32)
        nc.gpsimd.tensor_sub(dl[:], i12[:], i34[:])
        el = sm.tile([P, NT], F32)
        nc.gpsimd.tensor_mul(el[:], t1_b[:], dl[:])
        left = sm.tile([P, NT], F32)
        nc.gpsimd.tensor_add(left[:], el[:], i34[:])
        dr = sm.tile([P, NT], F32)
        nc.gpsimd.tensor_sub(dr[:], left[:], i56[:])
        er = sm.tile([P, NT], F32)
        nc.gpsimd.tensor_mul(er[:], rt_b[:], dr[:])
        cstar = sm.tile([P, NT], F32)
        nc.gpsimd.tensor_add(cstar[:], er[:], i56[:])

        nc.vector.memset(r_a[:], 0.0)
        ohc = sm.tile([P, NT], U8, name="ohc", bufs=2)
        for c, res in zip(range(2, 7), [radd, rsub, rmul, rdiv, rpow]):
            nc.vector.tensor_single_scalar(ohc[:], cstar[:], float(c),
                                           op=OP.is_equal)
            nc.vector.copy_predicated(r_a[:], ohc[:], res[:])

        nc.gpsimd.dma_start(io_d.ap().rearrange("(p q) -> p q", p=P), iffiv[:])

        # ---- td scatter emitted after both halves' compute so each engine's
        # in-order stream can overlap half0's r-wait with half1's work
        for h, (tdt, ohp0, rows) in enumerate(half_state):
            for t in range(TPH):
                ts = slice(t * S, (t + 1) * S)
                cidx = h * TPH + t
                fcol = fire_a[:, cidx : cidx + 1]
                rcol = r_a[:, cidx : cidx + 1]
                w0 = gsc.tile([P, S], U8, tag="w0", name="w0")
                nc.vector.tensor_scalar(w0[:], ohp0[:][:, ts], fcol, None,
                                        op0=OP.mult)
                rb = gsc.tile([P, S], F32, tag="rb", name="rb")
                nc.vector.tensor_scalar(rb[:], zero_bc[:], 0.0, rcol,
                                        op0=OP.mult, op1=OP.add)
                nc.vector.copy_predicated(tdt[:][:, ts], w0[:], rb[:])
            # one DMA per row-tile, alternating queues to drain in parallel
            for t in range(TPH):
                qout = nc.sync if (h * TPH + t) % 2 == 0 else nc.scalar
                rtile = slice((h * TPH + t) * P, (h * TPH + t + 1) * P)
                qout.dma_start(tdo_d[rtile, :],
                               tdt[:][:, t * S : (t + 1) * S])

    nc.compile()
    return nc


_NC_CACHE = {}


def _get_nc(sp_zero: bool = True):
    if sp_zero not in _NC_CACHE:
        _NC_CACHE[sp_zero] = _build_nc(sp_zero)
    return _NC_CACHE[sp_zero]


def _make_in_maps(trans_valid, trans_dense, trans_op, if_finished, if_valid,
                  start_pos):
    tv = np.ascontiguousarray(np.asarray(trans_valid, np.float32))
    td = np.ascontiguousarray(np.asarray(trans_dense, np.float32))
    # layout-only: channel planes [7, B, S]
    op = np.ascontiguousarray(
        np.asarray(trans_op, np.float32).transpose(2, 0, 1))
    fin = np.asarray(if_finished, np.float32)
    val = np.asarray(if_valid, np.float32)
    sp = int(start_pos)
    act2 = np.ascontiguousarray(
        np.tile((np.arange(S) >= sp).astype(np.float32), TPH))
    in_maps = []
    for c in range(NCORES):
        rows = slice(c * BS, (c + 1) * BS)
        # fv[p, t] = fin[t*128+p] ; fv[p, NT+t] = val[t*128+p]
        fvc = np.concatenate(
            [fin[rows].reshape(NT, P).T, val[rows].reshape(NT, P).T], axis=1)
        in_maps.append({
            "tv": tv[rows], "td": td[rows],
            "op": np.ascontiguousarray(op[:, rows, :]),
            "fv": np.ascontiguousarray(fvc), "act2": act2,
        })
    return in_maps


def _unpack_outs(outs, trans_op):
    tv_out = np.concatenate([outs[c]["tv_out"] for c in range(NCORES)], axis=0)
    td_out = np.concatenate([outs[c]["td_out"] for c in range(NCORES)], axis=0)
    iff = np.empty(B, np.float32)
    iv = np.empty(B, np.float32)
    for c in range(NCORES):
        arr = outs[c]["iffiv"].reshape(P, 2 * NT)
        rows = slice(c * BS, (c + 1) * BS)
        iff[rows] = arr[:, 0:NT].T.reshape(BS)
        iv[rows] = arr[:, NT : 2 * NT].T.reshape(BS)
    return tv_out, td_out, np.asarray(trans_op, np.float32), iff, iv


def kernel(trans_valid, trans_dense, trans_op, if_finished, if_valid, start_pos):
    nc = _get_nc(int(start_pos) == 0)
    in_maps = _make_in_maps(trans_valid, trans_dense, trans_op, if_finished,
                            if_valid, start_pos)
    res = run_bass_kernel_spmd(nc, in_maps, core_ids=list(range(NCORES)))
    return _unpack_outs(res.results, trans_op)


# revision 32
# speedup vs baseline: 1.1607x; 1.0013x over previous
"""Trainium2 Bass kernel for nn_ArthDenseCalcToDenseBlock.

The reference is a 256-step sequential scan per batch row, but the state
machine freezes at the first valid operator token (the `meet` gate), so the
whole scan collapses to closed-form masked reductions along the sequence
axis, computed per row with DVE prefix-scan instructions:

  mpre[j] = running-max of (valid-op mask)        -> first-op one-hot, met
  csuf[j] = reverse running-sum of (number mask)  -> last / 2nd-last number
                                                     one-hots via == 1 / == 2
  h0,h1 and the operator channel values are gathered with masked-sum
  accumulations; one predicated scatter writes the result back.

Data parallel over batch: 4096 rows -> 8 cores x 512 rows -> 2 halves of
[128, 2x256] merged tiles per core. trans_op is host-relayouted into 7
contiguous channel planes so every channel op is a contiguous 2D access.
Mask tensors are bf16 (0/1 and small counts are exact); trans_op values and
trans_dense stay f32 so argmax/select semantics match the reference
bit-exactly. Work is spread across DVE / GpSimd / ACT.
"""

from contextlib import ExitStack

import numpy as np

import concourse.bacc as bacc
import concourse.mybir as mybir
import concourse.tile as tile
from concourse.bass_utils import run_bass_kernel_spmd

F32 = mybir.dt.float32
BF16 = mybir.dt.bfloat16
U8 = mybir.dt.uint8
OP = mybir.AluOpType
ACTF = mybir.ActivationFunctionType

B, S, NOPS = 4096, 256, 7
NCORES = 8
BS = B // NCORES          # rows per core (512)
P = 128                   # partitions
NT = BS // P              # row-tiles per core (4)
NH = 2                    # halves per core
TPH = NT // NH            # row-tiles per half (2)
W = TPH * S               # free width of a merged half (512)


def _build_nc(sp_zero: bool):
    nc = bacc.Bacc("TRN2", target_bir_lowering=False, debug=False)

    tv_d = nc.dram_tensor("tv", [BS, S], F32, kind="ExternalInput")
    td_d = nc.dram_tensor("td", [BS, S], F32, kind="ExternalInput")
    # channel planes: op[c, row, s]
    op_d = nc.dram_tensor("op", [NOPS, BS, S], F32, kind="ExternalInput")
    fv_d = nc.dram_tensor("fv", [P, 2 * NT], F32, kind="ExternalInput")
    act_d = nc.dram_tensor("act2", [W], F32, kind="ExternalInput")

    tvo_d = nc.dram_tensor("tv_out", [BS, S], F32, kind="ExternalOutput")
    tdo_d = nc.dram_tensor("td_out", [BS, S], F32, kind="ExternalOutput")
    io_d = nc.dram_tensor("iffiv", [P * 2 * NT], F32, kind="ExternalOutput")

    with tile.TileContext(nc) as tc, ExitStack() as ctx:
        cpool = ctx.enter_context(tc.tile_pool(name="consts", bufs=1))
        io_pool = ctx.enter_context(tc.tile_pool(name="io", bufs=NH + 1))
        op_pool = ctx.enter_context(tc.tile_pool(name="op", bufs=2))
        work = ctx.enter_context(tc.tile_pool(name="work", bufs=2))
        gsc = ctx.enter_context(tc.tile_pool(name="gsc", bufs=12))
        sm = ctx.enter_context(tc.tile_pool(name="small", bufs=1))

        if not sp_zero:
            crow = cpool.tile([P, W], F32)
            nc.sync.dma_start(crow[0:1, :],
                              act_d.ap().rearrange("(o s) -> o s", o=1))
            actf = cpool.tile([P, W], F32)
            nc.gpsimd.partition_broadcast(actf[:], crow[0:1, :])
            act_bc = cpool.tile([P, W], BF16)
            nc.vector.tensor_copy(act_bc[:], actf[:])
        zero_bc = cpool.tile([P, S], F32)
        nc.vector.memset(zero_bc[:], 0.0)

        # ---- per-row gates [P, NT] (f cols 0..NT-1, g cols NT..2NT-1)
        fv = sm.tile([P, 2 * NT], F32)
        nc.gpsimd.dma_start(fv[:], fv_d[:, :])
        fin_t = fv[:, 0:NT]
        val_t = fv[:, NT : 2 * NT]
        omf = sm.tile([P, NT], F32)   # 1 - f
        nc.gpsimd.tensor_scalar(omf[:], fin_t, -1.0, 1.0, op0=OP.mult, op1=OP.add)
        gate = sm.tile([P, NT], F32)  # (1 - f) * g
        nc.gpsimd.tensor_mul(gate[:], omf[:], val_t)

        iffiv = sm.tile([P, 2 * NT], F32)
        # batched per-core scalars (columns = row-tile index 0..NT-1)
        h0_a = sm.tile([P, NT], F32)
        h1_a = sm.tile([P, NT], F32)
        enc_a = sm.tile([P, NT], F32)
        fire_a = sm.tile([P, NT], F32)
        r_a = sm.tile([P, NT], F32)

        half_state = []

        for h in range(NH):
            rows = slice(h * TPH * P, (h + 1) * TPH * P)
            tvt = io_pool.tile([P, W], F32, tag="tvt", name=f"tvt{h}")
            tdt = io_pool.tile([P, W], F32, tag="tdt", name=f"tdt{h}")
            chs = [None] * NOPS
            # pair-feeding channels on different queues so each tree leaf's
            # two inputs arrive in parallel
            qeng = {1: nc.sync, 2: nc.scalar, 3: nc.sync, 4: nc.scalar,
                    5: nc.sync, 6: nc.scalar, 0: nc.scalar}
            for c in [1, 2, 3, 4, 5, 6, 0]:
                cht = op_pool.tile([P, W], F32, tag=f"ch{c}", name=f"ch{c}_{h}")
                qeng[c].dma_start(
                    cht[:].rearrange("p (t s) -> p t s", t=TPH),
                    op_d[c, rows, :].rearrange("(t p) s -> p t s", p=P))
                chs[c] = cht
            nc.gpsimd.dma_start(
                tvt[:].rearrange("p (t s) -> p t s", t=TPH),
                tv_d[rows, :].rearrange("(t p) s -> p t s", p=P))
            nc.sync.dma_start(
                tdt[:].rearrange("p (t s) -> p t s", t=TPH),
                td_d[rows, :].rearrange("(t p) s -> p t s", p=P))

            # channel max over 1..6 via TT tree (DVE/ACT; Pool has no max)
            a1 = work.tile([P, W], F32, tag="a1", name=f"a1_{h}")
            nc.vector.tensor_max(a1[:], chs[1][:], chs[2][:])
            a2 = work.tile([P, W], F32, tag="a2", name=f"a2_{h}")
            nc.any.tensor_max(a2[:], chs[3][:], chs[4][:])
            a3 = work.tile([P, W], F32, tag="a3", name=f"a3_{h}")
            nc.any.tensor_max(a3[:], chs[5][:], chs[6][:])
            b1 = work.tile([P, W], F32, tag="b1", name=f"b1_{h}")
            nc.any.tensor_max(b1[:], a1[:], a2[:])
            m6 = work.tile([P, W], F32, tag="m6", name=f"m6_{h}")
            nc.vector.tensor_max(m6[:], b1[:], a3[:])
            is_op = work.tile([P, W], BF16, tag="isop", name=f"isop_{h}")
            nc.any.tensor_tensor(is_op[:], m6[:], chs[0][:], op=OP.is_gt)

            # argmax channel index, packed as 5 first-wins comparison bits
            s1 = work.tile([P, W], BF16, tag="s1", name=f"s1_{h}")
            nc.vector.tensor_tensor(s1[:], chs[1][:], chs[2][:], op=OP.is_ge)
            s2 = work.tile([P, W], BF16, tag="s2", name=f"s2_{h}")
            nc.any.tensor_tensor(s2[:], chs[3][:], chs[4][:], op=OP.is_ge)
            s3 = work.tile([P, W], BF16, tag="s3", name=f"s3_{h}")
            nc.any.tensor_tensor(s3[:], chs[5][:], chs[6][:], op=OP.is_ge)
            t1 = work.tile([P, W], BF16, tag="t1", name=f"t1_{h}")
            nc.any.tensor_tensor(t1[:], a1[:], a2[:], op=OP.is_ge)
            rt = work.tile([P, W], BF16, tag="rt", name=f"rt_{h}")
            nc.vector.tensor_tensor(rt[:], b1[:], a3[:], op=OP.is_ge)
            pk1 = work.tile([P, W], BF16, tag="pk1", name=f"pk1_{h}")
            nc.vector.scalar_tensor_tensor(
                pk1[:], s2[:], 2.0, s1[:], op0=OP.mult, op1=OP.add)
            pk2 = work.tile([P, W], BF16, tag="pk2", name=f"pk2_{h}")
            nc.vector.scalar_tensor_tensor(
                pk2[:], s3[:], 4.0, pk1[:], op0=OP.mult, op1=OP.add)
            pk3 = work.tile([P, W], BF16, tag="pk3", name=f"pk3_{h}")
            nc.vector.scalar_tensor_tensor(
                pk3[:], t1[:], 8.0, pk2[:], op0=OP.mult, op1=OP.add)
            enc = work.tile([P, W], BF16, tag="enc", name=f"enc_{h}")
            nc.vector.scalar_tensor_tensor(
                enc[:], rt[:], 16.0, pk3[:], op0=OP.mult, op1=OP.add)

            # valid-token mask (f/g gate folded into per-row scalars later)
            tvb = work.tile([P, W], BF16, tag="tvb", name=f"tvb_{h}")
            nc.vector.tensor_copy(tvb[:], tvt[:])
            if not sp_zero:
                tva = work.tile([P, W], BF16, tag="tva", name=f"tva_{h}")
                nc.vector.tensor_mul(tva[:], tvb[:], act_bc[:])
                tvb = tva
            vop = work.tile([P, W], BF16, tag="vop", name=f"vop_{h}")
            nc.vector.tensor_mul(vop[:], tvb[:], is_op[:])

            # mpre[j] = running max of vop (per 256-tile)
            mpre = work.tile([P, W], BF16, tag="mpre", name=f"mpre_{h}")
            for t in range(TPH):
                ts = slice(t * S, (t + 1) * S)
                nc.vector.tensor_tensor_scan(
                    mpre[:][:, ts], vop[:][:, ts], vop[:][:, ts], 0.0,
                    op0=OP.max, op1=OP.max)

            # nshx[1+j] = (mpre[j] != 1); nshx[0] = 1
            nshx = work.tile([P, W + 1], BF16, tag="nshx", name=f"nshx_{h}")
            nc.vector.tensor_single_scalar(
                nshx[:][:, 1 : W + 1], mpre[:], 1.0, op=OP.not_equal)
            nc.vector.memset(nshx[:][:, 0:1], 1.0)
            # first-op one-hot: vop & ~meet_exclusive (fix tile boundary col)
            ohi = work.tile([P, W], BF16, tag="ohi", name=f"ohi_{h}")
            nc.vector.tensor_mul(ohi[:], nshx[:][:, 0:W], vop[:])
            for t in range(1, TPH):
                nc.vector.tensor_copy(ohi[:][:, t * S : t * S + 1],
                                      vop[:][:, t * S : t * S + 1])
            # numbers before i*: tvb & ~mpre (inclusive)
            vnum = work.tile([P, W], BF16, tag="vnum", name=f"vnum_{h}")
            nc.vector.tensor_mul(vnum[:], nshx[:][:, 1 : W + 1], tvb[:])

            # csuf[j] = inclusive suffix count of vnum (reverse scan per tile)
            csuf = work.tile([P, W], BF16, tag="csuf", name=f"csuf_{h}")
            for t in range(TPH):
                ts = slice(t * S, (t + 1) * S)
                vr = vnum[:][:, ts][:, ::-1]
                nc.vector.tensor_tensor_scan(
                    csuf[:][:, ts][:, ::-1], vr, vr, 0.0, op0=OP.add, op1=OP.max)

            # last & second-to-last number one-hots (constant compares)
            e0 = work.tile([P, W], BF16, tag="e0", name=f"e0_{h}")
            nc.vector.tensor_single_scalar(e0[:], csuf[:], 1.0, op=OP.is_equal)
            ohp0 = work.tile([P, W], BF16, tag="ohp0", name=f"ohp0_{h}")
            nc.vector.tensor_mul(ohp0[:], e0[:], vnum[:])
            e1 = work.tile([P, W], BF16, tag="e1", name=f"e1_{h}")
            nc.vector.tensor_single_scalar(e1[:], csuf[:], 2.0, op=OP.is_equal)
            ohp1 = work.tile([P, W], BF16, tag="ohp1", name=f"ohp1_{h}")
            nc.vector.tensor_mul(ohp1[:], e1[:], vnum[:])

            # per-row scalars for this half
            met2 = mpre[:][:, S - 1 :: S]       # [P, TPH] bf16
            total2 = csuf[:][:, 0::S]           # [P, TPH] bf16
            gate2 = gate[:, h * TPH : (h + 1) * TPH]
            acols = slice(h * TPH, (h + 1) * TPH)

            cnt1 = sm.tile([P, TPH], F32, name=f"cnt1_{h}")
            nc.gpsimd.tensor_scalar(cnt1[:], total2, 0.5, None, op0=OP.is_gt)
            cnt2 = sm.tile([P, TPH], F32, name=f"cnt2_{h}")
            nc.gpsimd.tensor_scalar(cnt2[:], total2, 1.5, None, op0=OP.is_gt)
            metg = sm.tile([P, TPH], F32, name=f"metg_{h}")
            nc.gpsimd.tensor_mul(metg[:], met2, gate2)
            fire2 = fire_a[:, acols]
            nc.gpsimd.tensor_mul(fire2, metg[:], cnt2[:])

            # masked-sum gathers (exact: at most one nonzero term)
            def gather_dve(dst_col, src_ap, mask_ap):
                scr = gsc.tile([P, S], F32, tag="gscr", name="gscr")
                nc.vector.scalar_tensor_tensor(
                    scr[:], src_ap, 0.0, mask_ap, op0=OP.bypass, op1=OP.mult,
                    accum_out=dst_col)

            for t in range(TPH):
                ts = slice(t * S, (t + 1) * S)
                col = slice(h * TPH + t, h * TPH + t + 1)
                tds = tdt[:][:, ts]
                ohp0s, ohp1s, ohis = ohp0[:][:, ts], ohp1[:][:, ts], ohi[:][:, ts]
                gather_dve(h0_a[:, col], tds, ohp0s)
                gather_dve(h1_a[:, col], tds, ohp1s)
                gather_dve(enc_a[:, col], enc[:][:, ts], ohis)

            # iv / iff (gps smalls)
            nmet = sm.tile([P, TPH], F32, name=f"nmet_{h}")
            nc.gpsimd.tensor_scalar(nmet[:], metg[:], -1.0, 1.0,
                                    op0=OP.mult, op1=OP.add)
            g2 = val_t[:, acols]
            nmg = sm.tile([P, TPH], F32, name=f"nmg_{h}")
            nc.gpsimd.tensor_mul(nmg[:], nmet[:], g2)
            nc.gpsimd.tensor_add(iffiv[:, NT + h * TPH : NT + (h + 1) * TPH],
                                 fire2, nmg[:])
            ncnt2 = sm.tile([P, TPH], F32, name=f"ncnt2_{h}")
            nc.gpsimd.tensor_scalar(ncnt2[:], cnt2[:], -1.0, 1.0,
                                    op0=OP.mult, op1=OP.add)
            q1 = sm.tile([P, TPH], F32, name=f"q1_{h}")
            nc.gpsimd.tensor_mul(q1[:], nmg[:], cnt1[:])
            ifu = sm.tile([P, TPH], F32, name=f"ifu_{h}")
            nc.gpsimd.tensor_mul(ifu[:], q1[:], ncnt2[:])
            q2 = sm.tile([P, TPH], F32, name=f"q2_{h}")
            nc.gpsimd.tensor_mul(q2[:], omf[:, acols], ifu[:])
            nc.gpsimd.tensor_add(iffiv[:, h * TPH : (h + 1) * TPH],
                                 q2[:], fin_t[:, acols])

            # tv[p1] <- 0, tv[i*] <- 0 where fire  (doesn't need r)
            u1 = work.tile([P, W], BF16, tag="u1", name=f"u1_{h}")
            nc.gpsimd.tensor_add(u1[:], ohp1[:], ohi[:])
            for t in range(TPH):
                ts = slice(t * S, (t + 1) * S)
                fcol = fire_a[:, h * TPH + t : h * TPH + t + 1]
                nw1 = gsc.tile([P, S], F32, tag="nw1", name="nw1")
                nc.vector.tensor_scalar(nw1[:], u1[:][:, ts], fcol, 1.0,
                                        op0=OP.mult, op1=OP.not_equal)
                nc.vector.tensor_mul(tvt[:][:, ts], tvt[:][:, ts], nw1[:])
            nc.sync.dma_start(
                tvo_d[rows, :].rearrange("(t p) s -> p t s", p=P),
                tvt[:].rearrange("p (t s) -> p t s", t=TPH))

            half_state.append((tdt, ohp0, rows))

        # ---- batched r computation ([P, NT], one Ln + one Exp table load)
        radd = sm.tile([P, NT], F32)
        nc.gpsimd.tensor_add(radd[:], h1_a[:], h0_a[:])
        rsub = sm.tile([P, NT], F32)
        nc.gpsimd.tensor_sub(rsub[:], h1_a[:], h0_a[:])
        rmul = sm.tile([P, NT], F32)
        nc.gpsimd.tensor_mul(rmul[:], h1_a[:], h0_a[:])
        den = sm.tile([P, NT], F32)
        nc.gpsimd.tensor_scalar_add(den[:], h0_a[:], 1e-7)
        rec = sm.tile([P, NT], F32)
        nc.vector.reciprocal(rec[:], den[:])
        rdiv = sm.tile([P, NT], F32)
        nc.gpsimd.tensor_mul(rdiv[:], h1_a[:], rec[:])
        base = sm.tile([P, NT], F32)
        nc.gpsimd.tensor_scalar_max(base[:], h1_a[:], 1e-7)
        lg = sm.tile([P, NT], F32)
        nc.scalar.activation(lg[:], base[:], ACTF.Ln)
        pm = sm.tile([P, NT], F32)
        nc.gpsimd.tensor_mul(pm[:], lg[:], h0_a[:])
        rpow = sm.tile([P, NT], F32)
        nc.scalar.activation(rpow[:], pm[:], ACTF.Exp)

        def bit_split(src_t, thr, wgt, nlo, nbit):
            b = sm.tile([P, NT], F32, name=nbit)
            nc.gpsimd.tensor_scalar(b[:], src_t, thr, None, op0=OP.is_gt)
            m = sm.tile([P, NT], F32, name=nbit + "m")
            nc.gpsimd.tensor_scalar(m[:], b[:], wgt, None, op0=OP.mult)
            lo = sm.tile([P, NT], F32, name=nlo)
            nc.gpsimd.tensor_sub(lo[:], src_t, m[:])
            return b, lo

        rt_b, rr1 = bit_split(enc_a[:], 15.5, 16.0, "rr1", "rtb")
        t1_b, rr2 = bit_split(rr1[:], 7.5, 8.0, "rr2", "t1b")
        s3_b, rr3 = bit_split(rr2[:], 3.5, 4.0, "rr3", "s3b")
        s2_b, s1_b = bit_split(rr3[:], 1.5, 2.0, "s1b", "s2b")
        i12 = sm.tile([P, NT], F32)
        nc.gpsimd.tensor_scalar(i12[:], s1_b[:], -1.0, 2.0, op0=OP.mult, op1=OP.add)
        i34 = sm.tile([P, NT], F32)
        nc.gpsimd.tensor_scalar(i34[:], s2_b[:], -1.0, 4.0, op0=OP.mult, op1=OP.add)
        i56 = sm.tile([P, NT], F32)
        nc.gpsimd.tensor_scalar(i56[:], s3_b[:], -1.0, 6.0, op0=OP.mult, op1=OP.add)
        dl = sm.tile([P, NT], F32)
        nc.gpsimd.tensor_sub(dl[:], i12[:], i34[:])
        el = sm.tile([P, NT], F32)
        nc.gpsimd.tensor_mul(el[:], t1_b[:], dl[:])
        left = sm.tile([P, NT], F32)
        nc.gpsimd.tensor_add(left[:], el[:], i34[:])
        dr = sm.tile([P, NT], F32)
        nc.gpsimd.tensor_sub(dr[:], left[:], i56[:])
        er = sm.tile([P, NT], F32)
        nc.gpsimd.tensor_mul(er[:], rt_b[:], dr[:])
        cstar = sm.tile([P, NT], F32)
        nc.gpsimd.tensor_add(cstar[:], er[:], i56[:])

        nc.vector.memset(r_a[:], 0.0)
        ohc = sm.tile([P, NT], U8, name="ohc", bufs=2)
        for c, res in zip(range(2, 7), [radd, rsub, rmul, rdiv, rpow]):
            nc.vector.tensor_single_scalar(ohc[:], cstar[:], float(c),
                                           op=OP.is_equal)
            nc.vector.copy_predicated(r_a[:], ohc[:], res[:])

        nc.gpsimd.dma_start(io_d.ap().rearrange("(p q) -> p q", p=P), iffiv[:])

        # ---- td scatter emitted after both halves' compute so each engine's
        # in-order stream can overlap half0's r-wait with half1's work
        for h, (tdt, ohp0, rows) in enumerate(half_state):
            for t in range(TPH):
                ts = slice(t * S, (t + 1) * S)
                cidx = h * TPH + t
                fcol = fire_a[:, cidx : cidx + 1]
                rcol = r_a[:, cidx : cidx + 1]
                w0 = gsc.tile([P, S], U8, tag="w0", name="w0")
                nc.vector.tensor_scalar(w0[:], ohp0[:][:, ts], fcol, None,
                                        op0=OP.mult)
                rb = gsc.tile([P, S], F32, tag="rb", name="rb")
                nc.vector.tensor_scalar(rb[:], zero_bc[:], 0.0, rcol,
                                        op0=OP.mult, op1=OP.add)
                nc.vector.copy_predicated(tdt[:][:, ts], w0[:], rb[:])
            # one DMA per row-tile, alternating queues to drain in parallel
            for t in range(TPH):
                qout = nc.sync if (h * TPH + t) % 2 == 0 else nc.scalar
                rtile = slice((h * TPH + t) * P, (h * TPH + t + 1) * P)
                qout.dma_start(tdo_d[rtile, :],
                               tdt[:][:, t * S : (t + 1) * S])

    nc.compile()
    return nc


_NC_CACHE = {}


def _get_nc(sp_zero: bool = True):
    if sp_zero not in _NC_CACHE:
        _NC_CACHE[sp_zero] = _build_nc(sp_zero)
    return _NC_CACHE[sp_zero]


def _make_in_maps(trans_valid, trans_dense, trans_op, if_finished, if_valid,
                  start_pos):
    tv = np.ascontiguousarray(np.asarray(trans_valid, np.float32))
    td = np.ascontiguousarray(np.asarray(trans_dense, np.float32))
    # layout-only: channel planes [7, B, S]
    op = np.ascontiguousarray(
        np.asarray(trans_op, np.float32).transpose(2, 0, 1))
    fin = np.asarray(if_finished, np.float32)
    val = np.asarray(if_valid, np.float32)
    sp = int(start_pos)
    act2 = np.ascontiguousarray(
        np.tile((np.arange(S) >= sp).astype(np.float32), TPH))
    in_maps = []
    for c in range(NCORES):
        rows = slice(c * BS, (c + 1) * BS)
        # fv[p, t] = fin[t*128+p] ; fv[p, NT+t] = val[t*128+p]
        fvc = np.concatenate(
            [fin[rows].reshape(NT, P).T, val[rows].reshape(NT, P).T], axis=1)
        in_maps.append({
            "tv": tv[rows], "td": td[rows],
            "op": np.ascontiguousarray(op[:, rows, :]),
            "fv": np.ascontiguousarray(fvc), "act2": act2,
        })
    return in_maps


def _unpack_outs(outs, trans_op):
    tv_out = np.concatenate([outs[c]["tv_out"] for c in range(NCORES)], axis=0)
    td_out = np.concatenate([outs[c]["td_out"] for c in range(NCORES)], axis=0)
    iff = np.empty(B, np.float32)
    iv = np.empty(B, np.float32)
    for c in range(NCORES):
        arr = outs[c]["iffiv"].reshape(P, 2 * NT)
        rows = slice(c * BS, (c + 1) * BS)
        iff[rows] = arr[:, 0:NT].T.reshape(BS)
        iv[rows] = arr[:, NT : 2 * NT].T.reshape(BS)
    return tv_out, td_out, np.asarray(trans_op, np.float32), iff, iv


def kernel(trans_valid, trans_dense, trans_op, if_finished, if_valid, start_pos):
    nc = _get_nc(int(start_pos) == 0)
    in_maps = _make_in_maps(trans_valid, trans_dense, trans_op, if_finished,
                            if_valid, start_pos)
    res = run_bass_kernel_spmd(nc, in_maps, core_ids=list(range(NCORES)))
    return _unpack_outs(res.results, trans_op)


# revision 34
# speedup vs baseline: 1.1732x; 1.0108x over previous
"""Trainium2 Bass kernel for nn_ArthDenseCalcToDenseBlock.

The reference is a 256-step sequential scan per batch row, but the state
machine freezes at the first valid operator token (the `meet` gate), so the
whole scan collapses to closed-form masked reductions along the sequence
axis, computed per row with DVE prefix-scan instructions:

  mpre[j] = running-max of (valid-op mask)        -> first-op one-hot, met
  csuf[j] = reverse running-sum of (number mask)  -> last / 2nd-last number
                                                     one-hots via == 1 / == 2
  h0,h1 and the operator channel values are gathered with masked-sum
  accumulations; one predicated scatter writes the result back.

Data parallel over batch: 4096 rows -> 8 cores x 512 rows -> 2 halves of
[128, 2x256] merged tiles per core. trans_op is host-relayouted into 7
contiguous channel planes so every channel op is a contiguous 2D access.
Mask tensors are bf16 (0/1 and small counts are exact); trans_op values and
trans_dense stay f32 so argmax/select semantics match the reference
bit-exactly. Work is spread across DVE / GpSimd / ACT.
"""

from contextlib import ExitStack

import numpy as np

import concourse.bacc as bacc
import concourse.mybir as mybir
import concourse.tile as tile
from concourse.bass_utils import run_bass_kernel_spmd

F32 = mybir.dt.float32
BF16 = mybir.dt.bfloat16
U8 = mybir.dt.uint8
OP = mybir.AluOpType
ACTF = mybir.ActivationFunctionType

B, S, NOPS = 4096, 256, 7
NCORES = 8
BS = B // NCORES          # rows per core (512)
P = 128                   # partitions
NT = BS // P              # row-tiles per core (4)
NH = 2                    # halves per core
TPH = NT // NH            # row-tiles per half (2)
W = TPH * S               # free width of a merged half (512)


def _build_nc(sp_zero: bool):
    nc = bacc.Bacc("TRN2", target_bir_lowering=False, debug=False)

    tv_d = nc.dram_tensor("tv", [BS, S], F32, kind="ExternalInput")
    td_d = nc.dram_tensor("td", [BS, S], F32, kind="ExternalInput")
    # channel planes: op[c, row, s]
    op_d = nc.dram_tensor("op", [NOPS, BS, S], F32, kind="ExternalInput")
    fv_d = nc.dram_tensor("fv", [P, 2 * NT], F32, kind="ExternalInput")
    act_d = nc.dram_tensor("act2", [W], F32, kind="ExternalInput")

    tvo_d = nc.dram_tensor("tv_out", [BS, S], F32, kind="ExternalOutput")
    tdo_d = nc.dram_tensor("td_out", [BS, S], F32, kind="ExternalOutput")
    io_d = nc.dram_tensor("iffiv", [P * 2 * NT], F32, kind="ExternalOutput")

    with tile.TileContext(nc) as tc, ExitStack() as ctx:
        cpool = ctx.enter_context(tc.tile_pool(name="consts", bufs=1))
        io_pool = ctx.enter_context(tc.tile_pool(name="io", bufs=NH + 1))
        op_pool = ctx.enter_context(tc.tile_pool(name="op", bufs=2))
        work = ctx.enter_context(tc.tile_pool(name="work", bufs=2))
        gsc = ctx.enter_context(tc.tile_pool(name="gsc", bufs=12))
        sm = ctx.enter_context(tc.tile_pool(name="small", bufs=1))

        if not sp_zero:
            crow = cpool.tile([P, W], F32)
            nc.sync.dma_start(crow[0:1, :],
                              act_d.ap().rearrange("(o s) -> o s", o=1))
            actf = cpool.tile([P, W], F32)
            nc.gpsimd.partition_broadcast(actf[:], crow[0:1, :])
            act_bc = cpool.tile([P, W], BF16)
            nc.vector.tensor_copy(act_bc[:], actf[:])
        zero_bc = cpool.tile([P, S], F32)
        nc.vector.memset(zero_bc[:], 0.0)

        # ---- per-row gates [P, NT] (f cols 0..NT-1, g cols NT..2NT-1)
        fv = sm.tile([P, 2 * NT], F32)
        nc.gpsimd.dma_start(fv[:], fv_d[:, :])
        fin_t = fv[:, 0:NT]
        val_t = fv[:, NT : 2 * NT]
        omf = sm.tile([P, NT], F32)   # 1 - f
        nc.gpsimd.tensor_scalar(omf[:], fin_t, -1.0, 1.0, op0=OP.mult, op1=OP.add)
        gate = sm.tile([P, NT], F32)  # (1 - f) * g
        nc.gpsimd.tensor_mul(gate[:], omf[:], val_t)

        iffiv = sm.tile([P, 2 * NT], F32)
        # batched per-core scalars (columns = row-tile index 0..NT-1)
        h0_a = sm.tile([P, NT], F32)
        h1_a = sm.tile([P, NT], F32)
        enc_a = sm.tile([P, NT], F32)
        fire_a = sm.tile([P, NT], F32)
        r_a = sm.tile([P, NT], F32)

        half_state = []

        for h in range(NH):
            rows = slice(h * TPH * P, (h + 1) * TPH * P)
            tvt = io_pool.tile([P, W], F32, tag="tvt", name=f"tvt{h}")
            tdt = io_pool.tile([P, W], F32, tag="tdt", name=f"tdt{h}")
            chs = [None] * NOPS
            # pair-feeding channels on different queues so each tree leaf's
            # two inputs arrive in parallel
            qeng = {1: nc.sync, 2: nc.scalar, 3: nc.sync, 4: nc.scalar,
                    5: nc.sync, 6: nc.scalar, 0: nc.scalar}
            for c in [1, 2, 3, 4, 5, 6, 0]:
                cht = op_pool.tile([P, W], F32, tag=f"ch{c}", name=f"ch{c}_{h}")
                qeng[c].dma_start(
                    cht[:].rearrange("p (t s) -> p t s", t=TPH),
                    op_d[c, rows, :].rearrange("(t p) s -> p t s", p=P))
                chs[c] = cht
            nc.gpsimd.dma_start(
                tvt[:].rearrange("p (t s) -> p t s", t=TPH),
                tv_d[rows, :].rearrange("(t p) s -> p t s", p=P))
            nc.sync.dma_start(
                tdt[:].rearrange("p (t s) -> p t s", t=TPH),
                td_d[rows, :].rearrange("(t p) s -> p t s", p=P))

            # channel max over 1..6 via TT tree (DVE/ACT; Pool has no max)
            a1 = work.tile([P, W], F32, tag="a1", name=f"a1_{h}")
            nc.vector.tensor_max(a1[:], chs[1][:], chs[2][:])
            a2 = work.tile([P, W], F32, tag="a2", name=f"a2_{h}")
            nc.any.tensor_max(a2[:], chs[3][:], chs[4][:])
            a3 = work.tile([P, W], F32, tag="a3", name=f"a3_{h}")
            nc.any.tensor_max(a3[:], chs[5][:], chs[6][:])
            b1 = work.tile([P, W], F32, tag="b1", name=f"b1_{h}")
            nc.any.tensor_max(b1[:], a1[:], a2[:])
            m6 = work.tile([P, W], F32, tag="m6", name=f"m6_{h}")
            nc.vector.tensor_max(m6[:], b1[:], a3[:])
            is_op = work.tile([P, W], BF16, tag="isop", name=f"isop_{h}")
            nc.any.tensor_tensor(is_op[:], m6[:], chs[0][:], op=OP.is_gt)

            # argmax channel index, packed as 5 first-wins comparison bits
            s1 = work.tile([P, W], BF16, tag="s1", name=f"s1_{h}")
            nc.vector.tensor_tensor(s1[:], chs[1][:], chs[2][:], op=OP.is_ge)
            s2 = work.tile([P, W], BF16, tag="s2", name=f"s2_{h}")
            nc.any.tensor_tensor(s2[:], chs[3][:], chs[4][:], op=OP.is_ge)
            s3 = work.tile([P, W], BF16, tag="s3", name=f"s3_{h}")
            nc.any.tensor_tensor(s3[:], chs[5][:], chs[6][:], op=OP.is_ge)
            t1 = work.tile([P, W], BF16, tag="t1", name=f"t1_{h}")
            nc.any.tensor_tensor(t1[:], a1[:], a2[:], op=OP.is_ge)
            rt = work.tile([P, W], BF16, tag="rt", name=f"rt_{h}")
            nc.vector.tensor_tensor(rt[:], b1[:], a3[:], op=OP.is_ge)
            pk1 = work.tile([P, W], BF16, tag="pk1", name=f"pk1_{h}")
            nc.vector.scalar_tensor_tensor(
                pk1[:], s2[:], 2.0, s1[:], op0=OP.mult, op1=OP.add)
            pk2 = work.tile([P, W], BF16, tag="pk2", name=f"pk2_{h}")
            nc.vector.scalar_tensor_tensor(
                pk2[:], s3[:], 4.0, pk1[:], op0=OP.mult, op1=OP.add)
            pk3 = work.tile([P, W], BF16, tag="pk3", name=f"pk3_{h}")
            nc.vector.scalar_tensor_tensor(
                pk3[:], t1[:], 8.0, pk2[:], op0=OP.mult, op1=OP.add)
            enc = work.tile([P, W], BF16, tag="enc", name=f"enc_{h}")
            nc.vector.scalar_tensor_tensor(
                enc[:], rt[:], 16.0, pk3[:], op0=OP.mult, op1=OP.add)

            # valid-token mask (f/g gate folded into per-row scalars later)
            tvb = work.tile([P, W], BF16, tag="tvb", name=f"tvb_{h}")
            nc.vector.tensor_copy(tvb[:], tvt[:])
            if not sp_zero:
                tva = work.tile([P, W], BF16, tag="tva", name=f"tva_{h}")
                nc.vector.tensor_mul(tva[:], tvb[:], act_bc[:])
                tvb = tva
            vop = work.tile([P, W], BF16, tag="vop", name=f"vop_{h}")
            nc.vector.tensor_mul(vop[:], tvb[:], is_op[:])

            # mpre[j] = running max of vop (per 256-tile)
            mpre = work.tile([P, W], BF16, tag="mpre", name=f"mpre_{h}")
            for t in range(TPH):
                ts = slice(t * S, (t + 1) * S)
                nc.vector.tensor_tensor_scan(
                    mpre[:][:, ts], vop[:][:, ts], vop[:][:, ts], 0.0,
                    op0=OP.max, op1=OP.max)

            # nshx[1+j] = (mpre[j] != 1); nshx[0] = 1
            nshx = work.tile([P, W + 1], BF16, tag="nshx", name=f"nshx_{h}")
            nc.vector.tensor_single_scalar(
                nshx[:][:, 1 : W + 1], mpre[:], 1.0, op=OP.not_equal)
            nc.vector.memset(nshx[:][:, 0:1], 1.0)
            # first-op one-hot: vop & ~meet_exclusive (fix tile boundary col)
            ohi = work.tile([P, W], BF16, tag="ohi", name=f"ohi_{h}")
            nc.vector.tensor_mul(ohi[:], nshx[:][:, 0:W], vop[:])
            for t in range(1, TPH):
                nc.vector.tensor_copy(ohi[:][:, t * S : t * S + 1],
                                      vop[:][:, t * S : t * S + 1])
            # numbers before i*: tvb & ~mpre (inclusive)
            vnum = work.tile([P, W], BF16, tag="vnum", name=f"vnum_{h}")
            nc.vector.tensor_mul(vnum[:], nshx[:][:, 1 : W + 1], tvb[:])

            # csuf[j] = inclusive suffix count of vnum (reverse scan per tile)
            csuf = work.tile([P, W], BF16, tag="csuf", name=f"csuf_{h}")
            for t in range(TPH):
                ts = slice(t * S, (t + 1) * S)
                vr = vnum[:][:, ts][:, ::-1]
                nc.vector.tensor_tensor_scan(
                    csuf[:][:, ts][:, ::-1], vr, vr, 0.0, op0=OP.add, op1=OP.max)

            # last & second-to-last number one-hots (constant compares)
            e0 = work.tile([P, W], BF16, tag="e0", name=f"e0_{h}")
            nc.vector.tensor_single_scalar(e0[:], csuf[:], 1.0, op=OP.is_equal)
            ohp0 = work.tile([P, W], BF16, tag="ohp0", name=f"ohp0_{h}")
            nc.vector.tensor_mul(ohp0[:], e0[:], vnum[:])
            e1 = work.tile([P, W], BF16, tag="e1", name=f"e1_{h}")
            nc.vector.tensor_single_scalar(e1[:], csuf[:], 2.0, op=OP.is_equal)
            ohp1 = work.tile([P, W], BF16, tag="ohp1", name=f"ohp1_{h}")
            nc.vector.tensor_mul(ohp1[:], e1[:], vnum[:])

            # per-row scalars for this half
            met2 = mpre[:][:, S - 1 :: S]       # [P, TPH] bf16
            total2 = csuf[:][:, 0::S]           # [P, TPH] bf16
            gate2 = gate[:, h * TPH : (h + 1) * TPH]
            acols = slice(h * TPH, (h + 1) * TPH)

            cnt1 = sm.tile([P, TPH], F32, name=f"cnt1_{h}")
            nc.gpsimd.tensor_scalar(cnt1[:], total2, 0.5, None, op0=OP.is_gt)
            cnt2 = sm.tile([P, TPH], F32, name=f"cnt2_{h}")
            nc.gpsimd.tensor_scalar(cnt2[:], total2, 1.5, None, op0=OP.is_gt)
            metg = sm.tile([P, TPH], F32, name=f"metg_{h}")
            nc.gpsimd.tensor_mul(metg[:], met2, gate2)
            fire2 = fire_a[:, acols]
            nc.gpsimd.tensor_mul(fire2, metg[:], cnt2[:])

            # masked-sum gathers (exact: at most one nonzero term)
            def gather_dve(dst_col, src_ap, mask_ap):
                scr = gsc.tile([P, S], F32, tag="gscr", name="gscr")
                nc.vector.scalar_tensor_tensor(
                    scr[:], src_ap, 0.0, mask_ap, op0=OP.bypass, op1=OP.mult,
                    accum_out=dst_col)

            for t in range(TPH):
                ts = slice(t * S, (t + 1) * S)
                col = slice(h * TPH + t, h * TPH + t + 1)
                tds = tdt[:][:, ts]
                ohp0s, ohp1s, ohis = ohp0[:][:, ts], ohp1[:][:, ts], ohi[:][:, ts]
                gather_dve(h0_a[:, col], tds, ohp0s)
                gather_dve(h1_a[:, col], tds, ohp1s)
                gather_dve(enc_a[:, col], enc[:][:, ts], ohis)

            # iv / iff (gps smalls)
            nmet = sm.tile([P, TPH], F32, name=f"nmet_{h}")
            nc.gpsimd.tensor_scalar(nmet[:], metg[:], -1.0, 1.0,
                                    op0=OP.mult, op1=OP.add)
            g2 = val_t[:, acols]
            nmg = sm.tile([P, TPH], F32, name=f"nmg_{h}")
            nc.gpsimd.tensor_mul(nmg[:], nmet[:], g2)
            nc.gpsimd.tensor_add(iffiv[:, NT + h * TPH : NT + (h + 1) * TPH],
                                 fire2, nmg[:])
            ncnt2 = sm.tile([P, TPH], F32, name=f"ncnt2_{h}")
            nc.gpsimd.tensor_scalar(ncnt2[:], cnt2[:], -1.0, 1.0,
                                    op0=OP.mult, op1=OP.add)
            q1 = sm.tile([P, TPH], F32, name=f"q1_{h}")
            nc.gpsimd.tensor_mul(q1[:], nmg[:], cnt1[:])
            ifu = sm.tile([P, TPH], F32, name=f"ifu_{h}")
            nc.gpsimd.tensor_mul(ifu[:], q1[:], ncnt2[:])
            q2 = sm.tile([P, TPH], F32, name=f"q2_{h}")
            nc.gpsimd.tensor_mul(q2[:], omf[:, acols], ifu[:])
            nc.gpsimd.tensor_add(iffiv[:, h * TPH : (h + 1) * TPH],
                                 q2[:], fin_t[:, acols])

            # tv[p1] <- 0, tv[i*] <- 0 where fire  (doesn't need r)
            u1 = work.tile([P, W], BF16, tag="u1", name=f"u1_{h}")
            nc.gpsimd.tensor_add(u1[:], ohp1[:], ohi[:])
            for t in range(TPH):
                ts = slice(t * S, (t + 1) * S)
                fcol = fire_a[:, h * TPH + t : h * TPH + t + 1]
                nw1 = gsc.tile([P, S], F32, tag="nw1", name="nw1")
                nc.vector.tensor_scalar(nw1[:], u1[:][:, ts], fcol, 1.0,
                                        op0=OP.mult, op1=OP.not_equal)
                nc.vector.tensor_mul(tvt[:][:, ts], tvt[:][:, ts], nw1[:])
            nc.sync.dma_start(
                tvo_d[rows, :].rearrange("(t p) s -> p t s", p=P),
                tvt[:].rearrange("p (t s) -> p t s", t=TPH))

            half_state.append((tdt, ohp0, rows))

        # ---- batched r computation ([P, NT], one Ln + one Exp table load)
        radd = sm.tile([P, NT], F32)
        nc.gpsimd.tensor_add(radd[:], h1_a[:], h0_a[:])
        rsub = sm.tile([P, NT], F32)
        nc.gpsimd.tensor_sub(rsub[:], h1_a[:], h0_a[:])
        rmul = sm.tile([P, NT], F32)
        nc.gpsimd.tensor_mul(rmul[:], h1_a[:], h0_a[:])
        den = sm.tile([P, NT], F32)
        nc.gpsimd.tensor_scalar_add(den[:], h0_a[:], 1e-7)
        rec = sm.tile([P, NT], F32)
        nc.vector.reciprocal(rec[:], den[:])
        rdiv = sm.tile([P, NT], F32)
        nc.gpsimd.tensor_mul(rdiv[:], h1_a[:], rec[:])
        base = sm.tile([P, NT], F32)
        nc.gpsimd.tensor_scalar_max(base[:], h1_a[:], 1e-7)
        lg = sm.tile([P, NT], F32)
        nc.scalar.activation(lg[:], base[:], ACTF.Ln)
        pm = sm.tile([P, NT], F32)
        nc.gpsimd.tensor_mul(pm[:], lg[:], h0_a[:])
        rpow = sm.tile([P, NT], F32)
        nc.scalar.activation(rpow[:], pm[:], ACTF.Exp)

        def bit_split(src_t, thr, wgt, nlo, nbit):
            b = sm.tile([P, NT], F32, name=nbit)
            nc.gpsimd.tensor_scalar(b[:], src_t, thr, None, op0=OP.is_gt)
            m = sm.tile([P, NT], F32, name=nbit + "m")
            nc.gpsimd.tensor_scalar(m[:], b[:], wgt, None, op0=OP.mult)
            lo = sm.tile([P, NT], F32, name=nlo)
            nc.gpsimd.tensor_sub(lo[:], src_t, m[:])
            return b, lo

        rt_b, rr1 = bit_split(enc_a[:], 15.5, 16.0, "rr1", "rtb")
        t1_b, rr2 = bit_split(rr1[:], 7.5, 8.0, "rr2", "t1b")
        s3_b, rr3 = bit_split(rr2[:], 3.5, 4.0, "rr3", "s3b")
        s2_b, s1_b = bit_split(rr3[:], 1.5, 2.0, "s1b", "s2b")
        i12 = sm.tile([P, NT], F32)
        nc.gpsimd.tensor_scalar(i12[:], s1_b[:], -1.0, 2.0, op0=OP.mult, op1=OP.add)
        i34 = sm.tile([P, NT], F32)
        nc.gpsimd.tensor_scalar(i34[:], s2_b[:], -1.0, 4.0, op0=OP.mult, op1=OP.add)
        i56 = sm.tile([P, NT], F32)
        nc.gpsimd.tensor_scalar(i56[:], s3_b[:], -1.0, 6.0, op0=OP.mult, op1=OP.add)
        dl = sm.tile([P, NT], F32)
        nc.gpsimd.tensor_sub(dl[:], i12[:], i34[:])
        el = sm.tile([P, NT], F32)
        nc.gpsimd.tensor_mul(el[:], t1_b[:], dl[:])
        left = sm.tile([P, NT], F32)
        nc.gpsimd.tensor_add(left[:], el[:], i34[:])
        dr = sm.tile([P, NT], F32)
        nc.gpsimd.tensor_sub(dr[:], left[:], i56[:])
        er = sm.tile([P, NT], F32)
        nc.gpsimd.tensor_mul(er[:], rt_b[:], dr[:])
        cstar = sm.tile([P, NT], F32)
        nc.gpsimd.tensor_add(cstar[:], er[:], i56[:])

        nc.vector.memset(r_a[:], 0.0)
        ohc = sm.tile([P, NT], U8, name="ohc", bufs=2)
        for c, res in zip(range(2, 7), [radd, rsub, rmul, rdiv, rpow]):
            nc.vector.tensor_single_scalar(ohc[:], cstar[:], float(c),
                                           op=OP.is_equal)
            nc.vector.copy_predicated(r_a[:], ohc[:], res[:])

        nc.gpsimd.dma_start(io_d.ap().rearrange("(p q) -> p q", p=P), iffiv[:])

        # ---- td scatter emitted after both halves' compute so each engine's
        # in-order stream can overlap half0's r-wait with half1's work
        for h, (tdt, ohp0, rows) in enumerate(half_state):
            for t in range(TPH):
                ts = slice(t * S, (t + 1) * S)
                cidx = h * TPH + t
                fcol = fire_a[:, cidx : cidx + 1]
                rcol = r_a[:, cidx : cidx + 1]
                w0 = gsc.tile([P, S], U8, tag="w0", name="w0")
                nc.vector.tensor_scalar(w0[:], ohp0[:][:, ts], fcol, None,
                                        op0=OP.mult)
                rb = gsc.tile([P, S], F32, tag="rb", name="rb")
                nc.vector.tensor_scalar(rb[:], zero_bc[:], 0.0, rcol,
                                        op0=OP.mult, op1=OP.add)
                nc.vector.copy_predicated(tdt[:][:, ts], w0[:], rb[:])
            # one DMA per row-tile, alternating queues to drain in parallel
            for t in range(TPH):
                qout = nc.sync if (h * TPH + t) % 2 == 0 else nc.scalar
                rtile = slice((h * TPH + t) * P, (h * TPH + t + 1) * P)
                qout.dma_start(tdo_d[rtile, :],
                               tdt[:][:, t * S : (t + 1) * S])

    nc.compile()
    return nc


_NC_CACHE = {}


def _get_nc(sp_zero: bool = True):
    if sp_zero not in _NC_CACHE:
        _NC_CACHE[sp_zero] = _build_nc(sp_zero)
    return _NC_CACHE[sp_zero]


def _make_in_maps(trans_valid, trans_dense, trans_op, if_finished, if_valid,
                  start_pos):
    tv = np.ascontiguousarray(np.asarray(trans_valid, np.float32))
    td = np.ascontiguousarray(np.asarray(trans_dense, np.float32))
    # layout-only: channel planes [7, B, S]
    op = np.ascontiguousarray(
        np.asarray(trans_op, np.float32).transpose(2, 0, 1))
    fin = np.asarray(if_finished, np.float32)
    val = np.asarray(if_valid, np.float32)
    sp = int(start_pos)
    act2 = np.ascontiguousarray(
        np.tile((np.arange(S) >= sp).astype(np.float32), TPH))
    in_maps = []
    for c in range(NCORES):
        rows = slice(c * BS, (c + 1) * BS)
        # fv[p, t] = fin[t*128+p] ; fv[p, NT+t] = val[t*128+p]
        fvc = np.concatenate(
            [fin[rows].reshape(NT, P).T, val[rows].reshape(NT, P).T], axis=1)
        in_maps.append({
            "tv": tv[rows], "td": td[rows],
            "op": np.ascontiguousarray(op[:, rows, :]),
            "fv": np.ascontiguousarray(fvc), "act2": act2,
        })
    return in_maps


def _unpack_outs(outs, trans_op):
    tv_out = np.concatenate([outs[c]["tv_out"] for c in range(NCORES)], axis=0)
    td_out = np.concatenate([outs[c]["td_out"] for c in range(NCORES)], axis=0)
    iff = np.empty(B, np.float32)
    iv = np.empty(B, np.float32)
    for c in range(NCORES):
        arr = outs[c]["iffiv"].reshape(P, 2 * NT)
        rows = slice(c * BS, (c + 1) * BS)
        iff[rows] = arr[:, 0:NT].T.reshape(BS)
        iv[rows] = arr[:, NT : 2 * NT].T.reshape(BS)
    return tv_out, td_out, np.asarray(trans_op, np.float32), iff, iv


def kernel(trans_valid, trans_dense, trans_op, if_finished, if_valid, start_pos):
    nc = _get_nc(int(start_pos) == 0)
    in_maps = _make_in_maps(trans_valid, trans_dense, trans_op, if_finished,
                            if_valid, start_pos)
    res = run_bass_kernel_spmd(nc, in_maps, core_ids=list(range(NCORES)))
    return _unpack_outs(res.results, trans_op)
